# revision 9
# baseline (speedup 1.0000x reference)
"""EdgeConv2d (gnn_message_passing) Trainium2 Bass kernel.

Reference computation (B=2, C=64, N=32768, K=16, OUT=64):
    xf  = x[..., 0]                               # [B, C, N]
    x_i = xf[:, :, edge_index[1]]                 # [B, C, N, K]
    x_j = xf[:, :, edge_index[0]]
    y   = W @ [x_i ; x_j - x_i] + b               # [B, OUT, N, K]
    y   = batchnorm_train(y) * gamma + beta ; relu
    out = max_k y                                 # [B, OUT, N, 1]

Device strategy (8 NeuronCores, nodes sharded):
    W @ [x_i; x_j - x_i] = (W1-W2) @ x_i + W2 @ x_j, so precompute node
    tables T1 = x^T (W1-W2)^T, T2 = x^T W2^T once per node; the per-edge
    work collapses to  y[e] = T1[idx1[e]] + T2[idx0[e]]  (gather + add).
    BatchNorm then max_k commutes through the per-channel affine:
        max_k relu(a*y+c) = relu(max(a*ymax+c, a*ymin+c))
    so kernel B only needs per-(node,ch) ymax/ymin over K plus global
    per-channel sum(y^2) (sum(y) is linear => host computes it from T and
    an index histogram).

    Kernel A: sharded table build   (x slice -> T1/T2 slices, tiny matmuls)
    Kernel B: dma_gather from full T tables + add + min/max + sum(y^2)
    Kernel C: per-channel affine + relu + PE transpose to channel-major
    Host: stats reduction between B and C (O(64) work).
"""

import sys

for _p in ("/opt/trn_rl_repo", "/root/.axon_site/_ro/trn_rl_repo"):
    if _p not in sys.path:
        sys.path.insert(0, _p)

from contextlib import ExitStack

import numpy as np

import concourse.bass as bass
import concourse.tile as tile
from concourse import bacc, mybir
from concourse.bass_utils import run_bass_kernel_spmd
from concourse.masks import make_identity

F32 = mybir.dt.float32
I16 = mybir.dt.int16

B, C, N, K, OUT = 2, 64, 32768, 16, 64
EPS = 1e-5
NCORES = 8
NS = N // NCORES          # nodes per core per batch (4096)
TILES = NS // 128         # 128-node tiles per batch per core (32)
TWO_C = 2 * C             # 128
E_TOT = B * N * K         # total edges (BN population size)

_PROG_CACHE = {}
NQ = 4
SINGLE_PACKET = False
LAST_RESULTS = {}  # debug: kernel-stage name -> BassKernelResults


def _run(nc, in_maps, cores, tag):
    import os

    trace = os.environ.get("KERNEL_TRACE", "0") == "1"
    r = run_bass_kernel_spmd(nc, in_maps, core_ids=cores, trace=trace)
    LAST_RESULTS[tag] = r
    return r.results


def _new_nc():
    return bacc.Bacc(
        "TRN2",
        target_bir_lowering=False,
        debug=False,
        enable_asserts=True,
        num_devices=NCORES,
        num_swdge_queues=NQ,
    )


# --------------------------------------------------------------------------
# Kernel A: per-core T-table build.
#   in : xs [B, C, NS] f32 (this core's node slice), u [C, 2C] f32
#        (u = [U1^T | U2^T], U1 = W1-W2, U2 = W2)
#   out: t1 [B, NS, OUT] f32, t2 [B, NS, OUT] f32   (node-major rows)
# --------------------------------------------------------------------------
def _build_kernel_a():
    nc = _new_nc()
    xs = nc.dram_tensor("xs", [B, C, NS], F32, kind="ExternalInput").ap()
    u = nc.dram_tensor("u", [C, TWO_C], F32, kind="ExternalInput").ap()
    t1 = nc.dram_tensor("t1", [B, NS, OUT], F32, kind="ExternalOutput").ap()
    t2 = nc.dram_tensor("t2", [B, NS, OUT], F32, kind="ExternalOutput").ap()

    with tile.TileContext(nc) as tc, ExitStack() as ctx:
        const = ctx.enter_context(tc.tile_pool(name="const", bufs=1))
        xin = ctx.enter_context(tc.tile_pool(name="xin", bufs=2))
        stg = ctx.enter_context(tc.tile_pool(name="stg", bufs=2))
        pp = ctx.enter_context(tc.tile_pool(name="pp", bufs=4, space="PSUM"))

        ut = const.tile([C, TWO_C], F32)
        nc.sync.dma_start(ut[:], u[:, :])

        for b in range(B):
            xb = xin.tile([C, NS], F32)
            nc.sync.dma_start(xb[:], xs[b])
            s12 = stg.tile([128, TILES * TWO_C], F32)
            for t in range(TILES):
                ps = pp.tile([128, TWO_C], F32)
                nc.tensor.matmul(ps[:], lhsT=xb[:, t * 128:(t + 1) * 128],
                                 rhs=ut[:], start=True, stop=True)
                nc.vector.tensor_copy(
                    s12[:, t * TWO_C:(t + 1) * TWO_C], ps[:]
                )
            # stage layout [p, t*128 + (T1 o | T2 o)] -> dram row n = t*128+p
            sv = s12[:].rearrange("p (t g o) -> p g t o", g=2, o=OUT)
            d1 = t1[b].rearrange("(t p) o -> p t o", p=128)
            d2 = t2[b].rearrange("(t p) o -> p t o", p=128)
            nc.sync.dma_start(d1, sv[:, 0])
            nc.sync.dma_start(d2, sv[:, 1])
    nc.compile()
    return nc


# --------------------------------------------------------------------------
# Kernel B: gather + add + minmax + sum(y^2).
#   in : tb<b>_<1|2> [N, OUT] f32 (full tables), idx [B, TILES, 2, 128, 128] i16
#   out: ymax, ymin [B, TILES, 128, OUT] f32, s2 [OUT, 1] f32
# idx[b,t,0] gathers T1 rows (edge_index[1]); idx[b,t,1] gathers T2 rows
# (edge_index[0]); index order i = k*128 + p so node p's K edges land in
# free-dim slots of partition p.
# --------------------------------------------------------------------------
def _build_kernel_b():
    nc = _new_nc()
    tabs = [
        [
            nc.dram_tensor(f"tb{b}_{g}", [N, OUT], F32, kind="ExternalInput").ap()
            for g in (1, 2)
        ]
        for b in range(B)
    ]
    idx = nc.dram_tensor(
        "idx", [128, B * TILES * 2 * 128], I16, kind="ExternalInput"
    ).ap()
    ymax = nc.dram_tensor(
        "ymax", [B, TILES, 128, OUT], F32, kind="ExternalOutput"
    ).ap()
    ymin = nc.dram_tensor(
        "ymin", [B, TILES, 128, OUT], F32, kind="ExternalOutput"
    ).ap()
    s2 = nc.dram_tensor("s2", [OUT, 1], F32, kind="ExternalOutput").ap()

    NI = 2048  # indices per gather (128 nodes x K)

    with tile.TileContext(nc) as tc, ExitStack() as ctx:
        accp = ctx.enter_context(tc.tile_pool(name="accp", bufs=1))
        gp = ctx.enter_context(tc.tile_pool(name="gp", bufs=3))
        yp = ctx.enter_context(tc.tile_pool(name="yp", bufs=2))
        sqp = ctx.enter_context(tc.tile_pool(name="sqp", bufs=2))
        redp = ctx.enter_context(tc.tile_pool(name="redp", bufs=2))
        stg = ctx.enter_context(tc.tile_pool(name="stg", bufs=2))
        outp = ctx.enter_context(tc.tile_pool(name="outp", bufs=1))
        pp = ctx.enter_context(tc.tile_pool(name="pp", bufs=1, space="PSUM"))

        sqacc = accp.tile([128, OUT], F32)
        ones = accp.tile([128, 1], F32)
        nc.vector.memset(sqacc[:], 0.0)
        nc.vector.memset(ones[:], 1.0)
        idxall = accp.tile([128, B * TILES * 2 * 128], I16)
        nc.sync.dma_start(idxall[:], idx[:, :])

        for b in range(B):
            smax = stg.tile([128, TILES * OUT], F32, tag="smax")
            smin = stg.tile([128, TILES * OUT], F32, tag="smin")
            for t in range(TILES):
                j = ((b * TILES + t) * 2) * 128
                g1 = gp.tile([128, K, OUT], F32, tag="g1")
                g2 = gp.tile([128, K, OUT], F32, tag="g2")
                nc.gpsimd.dma_gather(
                    g1[:], tabs[b][0][:, :], idxall[:, j:j + 128], NI, NI, OUT,
                    queue_num=(2 * t) % NQ, single_packet=SINGLE_PACKET,
                )
                nc.gpsimd.dma_gather(
                    g2[:], tabs[b][1][:, :], idxall[:, j + 128:j + 256], NI, NI,
                    OUT, queue_num=(2 * t + 1) % NQ, single_packet=SINGLE_PACKET,
                )
                y = yp.tile([128, K, OUT], F32)
                nc.vector.tensor_add(
                    y[:].rearrange("p k c -> p (k c)"),
                    g1[:].rearrange("p k c -> p (k c)"),
                    g2[:].rearrange("p k c -> p (k c)"),
                )
                ysq = sqp.tile([128, K, OUT], F32)
                nc.scalar.activation(
                    ysq[:].rearrange("p k c -> p (k c)"),
                    y[:].rearrange("p k c -> p (k c)"),
                    mybir.ActivationFunctionType.Square,
                )
                yv = y[:].rearrange("p k c -> p c k")
                nc.vector.tensor_reduce(
                    smax[:, t * OUT:(t + 1) * OUT],
                    yv, axis=mybir.AxisListType.X, op=mybir.AluOpType.max,
                )
                nc.vector.tensor_reduce(
                    smin[:, t * OUT:(t + 1) * OUT],
                    yv, axis=mybir.AxisListType.X, op=mybir.AluOpType.min,
                )
                sq = redp.tile([128, OUT], F32)
                nc.vector.tensor_reduce(
                    sq[:], ysq[:].rearrange("p k c -> p c k"),
                    axis=mybir.AxisListType.X, op=mybir.AluOpType.add,
                )
                nc.vector.tensor_add(sqacc[:], sqacc[:], sq[:])
            nc.sync.dma_start(
                ymax[b].rearrange("t p o -> p t o"),
                smax[:].rearrange("p (t o) -> p t o", o=OUT),
            )
            nc.sync.dma_start(
                ymin[b].rearrange("t p o -> p t o"),
                smin[:].rearrange("p (t o) -> p t o", o=OUT),
            )
        # partition-reduce sqacc [128, OUT] -> [OUT, 1]
        ps = pp.tile([OUT, 1], F32)
        nc.tensor.matmul(ps[:], lhsT=sqacc[:], rhs=ones[:], start=True, stop=True)
        s2sb = outp.tile([OUT, 1], F32)
        nc.vector.tensor_copy(s2sb[:], ps[:])
        nc.sync.dma_start(s2[:, :], s2sb[:])
    nc.compile()
    return nc


# --------------------------------------------------------------------------
# Kernel C: finalize.  out = relu(max(a*ymax+c, a*ymin+c)), transposed to
# channel-major.
#   in : ymax, ymin [B, TILES, 128, OUT] f32, ac [OUT, 2] f32 (a | c)
#   out: yout [B, OUT, NS] f32
# --------------------------------------------------------------------------
def _build_kernel_c():
    nc = _new_nc()
    ymax = nc.dram_tensor(
        "ymax", [B, TILES, 128, OUT], F32, kind="ExternalInput"
    ).ap()
    ymin = nc.dram_tensor(
        "ymin", [B, TILES, 128, OUT], F32, kind="ExternalInput"
    ).ap()
    ac = nc.dram_tensor("ac", [128, 2], F32, kind="ExternalInput").ap()
    yout = nc.dram_tensor("yout", [B, OUT, NS], F32, kind="ExternalOutput").ap()

    with tile.TileContext(nc) as tc, ExitStack() as ctx:
        const = ctx.enter_context(tc.tile_pool(name="const", bufs=1))
        ld = ctx.enter_context(tc.tile_pool(name="ld", bufs=2))
        tmp = ctx.enter_context(tc.tile_pool(name="tmp", bufs=3))
        ostg = ctx.enter_context(tc.tile_pool(name="ostg", bufs=2))
        pp = ctx.enter_context(tc.tile_pool(name="pp", bufs=4, space="PSUM"))

        ident = const.tile([128, 128], F32)
        make_identity(nc, ident[:])
        act = const.tile([128, 2], F32)
        nc.sync.dma_start(act[:], ac[:, :])

        for b in range(B):
            lmax = ld.tile([128, TILES * OUT], F32, tag="lmax")
            lmin = ld.tile([128, TILES * OUT], F32, tag="lmin")
            nc.sync.dma_start(
                lmax[:].rearrange("p (t o) -> p t o", o=OUT),
                ymax[b].rearrange("t p o -> p t o"),
            )
            nc.sync.dma_start(
                lmin[:].rearrange("p (t o) -> p t o", o=OUT),
                ymin[b].rearrange("t p o -> p t o"),
            )
            # transpose tile PAIRS: psum rows 0:64 = tile 2q channels,
            # rows 64:128 = tile 2q+1 channels (ac rows are duplicated).
            ob = ostg.tile([128, NS // 2], F32)
            for q in range(TILES // 2):
                pmax = pp.tile([128, 128], F32, tag="pmax")
                nc.tensor.transpose(
                    pmax[:], lmax[:, q * 128:(q + 1) * 128], ident[:]
                )
                pmin = pp.tile([128, 128], F32, tag="pmin")
                nc.tensor.transpose(
                    pmin[:], lmin[:, q * 128:(q + 1) * 128], ident[:]
                )
                m1 = tmp.tile([128, 128], F32, tag="m1")
                nc.scalar.activation(
                    m1[:], pmax[:], mybir.ActivationFunctionType.Identity,
                    bias=act[:, 1:2], scale=act[:, 0:1],
                )
                m2 = tmp.tile([128, 128], F32, tag="m2")
                nc.vector.tensor_scalar(
                    m2[:], pmin[:], act[:, 0:1], act[:, 1:2],
                    op0=mybir.AluOpType.mult, op1=mybir.AluOpType.add,
                )
                nc.vector.tensor_tensor(
                    ob[:, q * 128:(q + 1) * 128], m1[:], m2[:],
                    op=mybir.AluOpType.max,
                )
            nc.scalar.activation(
                ob[:], ob[:], mybir.ActivationFunctionType.Relu
            )
            # parity-split store: partitions 0:64 are even tiles, 64:128 odd
            dv = yout[b].rearrange(
                "o (q par col) -> par o q col", par=2, col=128
            )
            nc.sync.dma_start(
                dv[0], ob[0:OUT].rearrange("p (q col) -> p q col", col=128)
            )
            nc.sync.dma_start(
                dv[1], ob[OUT:128].rearrange("p (q col) -> p q col", col=128)
            )
    nc.compile()
    return nc


def _get_progs():
    if "a" not in _PROG_CACHE:
        _PROG_CACHE["a"] = _build_kernel_a()
        _PROG_CACHE["b"] = _build_kernel_b()
        _PROG_CACHE["c"] = _build_kernel_c()
    return _PROG_CACHE["a"], _PROG_CACHE["b"], _PROG_CACHE["c"]


def _prep_indices(ei):
    """edge_index [2, B, N, K] -> per-core int16 gather indices
    [NCORES, 128, B*TILES*2*128] (partition-major, contiguous per partition).
    Gathered row i of block (b,t,g) comes from partition i % 16 (replicated
    8x over the 128 partitions), column i // 16; i = k*128 + p_node."""
    e = ei.reshape(2, B, NCORES, TILES, 128, K)
    e = np.stack([e[1], e[0]], axis=3)  # [B, NCORES, TILES, 2, 128(p), K]
    flat = e.transpose(1, 0, 2, 3, 5, 4).reshape(NCORES, B, TILES, 2, K * 128)
    arr = flat.reshape(NCORES, B, TILES, 2, 128, 16).transpose(0, 1, 2, 3, 5, 4)
    rep = np.tile(arr, (1, 1, 1, 1, 8, 1))  # [NCORES, B, TILES, 2, 128, 128]
    # -> [NCORES, 128(part), B, TILES, 2, 128(s)]
    rep = rep.transpose(0, 4, 1, 2, 3, 5).reshape(NCORES, 128, -1)
    return np.ascontiguousarray(rep.astype(np.int16))


def kernel(x, edge_index, W, b, gamma, beta):
    x = np.asarray(x, dtype=np.float32)
    ei = np.asarray(edge_index)
    W = np.asarray(W, dtype=np.float32)
    bb = np.asarray(b, dtype=np.float64)
    gamma = np.asarray(gamma, dtype=np.float64)
    beta = np.asarray(beta, dtype=np.float64)

    nc_a, nc_b, nc_c = _get_progs()
    cores = list(range(NCORES))

    xf = np.ascontiguousarray(x[..., 0])  # [B, C, N]
    W1, W2 = W[:, :C], W[:, C:]
    u = np.ascontiguousarray(
        np.concatenate([(W1 - W2).T, W2.T], axis=1)
    )  # [C, 2C]

    # ---- Kernel A: build tables ----
    in_a = [
        {
            "xs": np.ascontiguousarray(xf[:, :, c * NS:(c + 1) * NS]),
            "u": u,
        }
        for c in cores
    ]
    res_a = _run(nc_a, in_a, cores, "a")
    t1 = np.concatenate([r["t1"] for r in res_a], axis=1)  # [B, N, OUT]
    t2 = np.concatenate([r["t2"] for r in res_a], axis=1)

    # ---- host: linear part of the stats ----
    s1 = np.zeros(OUT, np.float64)
    for bi in range(B):
        c1 = np.bincount(ei[1, bi].ravel(), minlength=N).astype(np.float64)
        c0 = np.bincount(ei[0, bi].ravel(), minlength=N).astype(np.float64)
        s1 += c1 @ t1[bi].astype(np.float64) + c0 @ t2[bi].astype(np.float64)

    # ---- Kernel B: gather + minmax + sum(y^2) ----
    idx16 = _prep_indices(ei)
    in_b = [
        {
            "tb0_1": np.ascontiguousarray(t1[0]),
            "tb0_2": np.ascontiguousarray(t2[0]),
            "tb1_1": np.ascontiguousarray(t1[1]),
            "tb1_2": np.ascontiguousarray(t2[1]),
            "idx": idx16[c],
        }
        for c in cores
    ]
    res_b = _run(nc_b, in_b, cores, "b")

    s2 = sum(r["s2"][:, 0].astype(np.float64) for r in res_b)

    # ---- host: batch-norm affine (bias folded in) ----
    e_tot = float(E_TOT)
    s1t = s1 + e_tot * bb
    s2t = s2 + 2.0 * bb * s1 + e_tot * bb * bb
    mean = s1t / e_tot
    var = s2t / e_tot - mean * mean
    a_coef = gamma / np.sqrt(var + EPS)
    c_coef = beta + (bb - mean) * a_coef
    ac = np.ascontiguousarray(
        np.tile(np.stack([a_coef, c_coef], axis=1).astype(np.float32), (2, 1))
    )

    # ---- Kernel C: finalize ----
    in_c = [
        {"ymax": res_b[c]["ymax"], "ymin": res_b[c]["ymin"], "ac": ac}
        for c in cores
    ]
    res_c = _run(nc_c, in_c, cores, "c")

    out = np.concatenate([r["yout"] for r in res_c], axis=2)  # [B, OUT, N]
    return np.ascontiguousarray(out[..., None]).astype(np.float32)
